# revision 1
# baseline (speedup 1.0000x reference)
"""Trainium2 Bass kernel for the nn_Points problem.

Renders N=1024 anisotropic "diamond" points onto a 3x256x384 canvas:
    t = (pixel - loc) @ M_n          (2-vector per pixel per point)
    mapped = relu(1 - (|t0|+|t1|)/2)
    canvas = sigmoid(4 * sum_n mapped * color_n)

Strategy (8 NeuronCores, full inputs in / full output out):
  * Spatial-shard the canvas: core c renders rows [32c, 32c+32).
  * Within a core: 24 spatial tiles of 4 rows x 128 cols (512 px).
  * Host-side exact culling: point n can touch a tile only if
    sigma_min(M_n) * dist2(loc_n, tile_rect) <= 2  (else |t|_1 >= 2
    everywhere in the tile and mapped is exactly 0).  Measured <= ~82
    points per tile, so one 128-slot point tile per spatial tile.
  * u = t0+t1, v = t0-t1 are affine in (gy, gx) -> computed as K=8
    fp16 matmuls (hi/lo split of coords/consts for fp32-grade accuracy):
        out[pt, px] = W[k, pt].T @ G[k, px]
  * |t0|+|t1| = max(|u|,|v|): one ACT Abs over both PSUM banks, one DVE
    fp16 max, then mapped'' = min(d,2)-2 (= -2*relu(1-d/2)) as a fused
    DVE tensor_scalar; the -0.5 sign/scale is folded into the colors.
  * canvas: matmul with mapped (fp16, SBUF) as the stationary operand,
    colors [128pts, 3] as moving operand; accumulates [128px, 3] blocks
    into one persistent PSUM bank laid out [128, 32rows*3blk*3ch].
  * One sigmoid(4x) ACT over the whole core's canvas + one DMA out.
"""

import math
import os
import sys

import numpy as np

for _p in ("/opt/trn_rl_repo",):
    if _p not in sys.path and os.path.isdir(_p):
        sys.path.insert(0, _p)

# Geometry (matches the reference module's fixed canvas).
H, W = 256, 384
N_CORES = 8
ROWS_PER_CORE = H // N_CORES            # 32
TILE_ROWS, TILE_COLS = 4, 128           # spatial tile = 512 px
N_BANDS = ROWS_PER_CORE // TILE_ROWS    # 8 row-bands per core
N_BLOCKS = W // TILE_COLS               # 3 col-blocks per row
TILES_PER_CORE = N_BANDS * N_BLOCKS     # 24
TILE_PX = TILE_ROWS * TILE_COLS         # 512
CAP = 128                               # points per point-tile
WIDTH_TO_HEIGHT = 384.0 / 256.0

# Set BASS_KERNEL_TRACE=1 to capture an NTFF profile; results land here.
last_run_info = {}


def _hi_lo(x):
    """Split float64 array into fp16 hi + fp16 lo with tiny residual."""
    hi = x.astype(np.float16)
    lo = (x - hi.astype(np.float64)).astype(np.float16)
    return hi, lo


def _prepare(locations, matrix_offsets, matrix_scale_exponents, colors):
    """Host-side prep: per-point combos, culling, per-core packed arrays."""
    loc = np.asarray(locations, np.float64).reshape(-1, 2)      # (N, 2) y,x
    mo = np.asarray(matrix_offsets, np.float64)                  # (N, 2, 2)
    mse = np.asarray(matrix_scale_exponents, np.float64).reshape(-1)
    cols = np.asarray(colors, np.float64).reshape(-1, 3)         # (N, 3)
    n = loc.shape[0]

    scale = (math.sqrt(n) / 2.0) / np.exp(mse)
    mats = mo + np.eye(2)[None, :, :] * scale[:, None, None]     # (N, 2, 2)
    # b_j = loc_y*M[0,j] + loc_x*M[1,j]
    b = loc[:, 0, None] * mats[:, 0, :] + loc[:, 1, None] * mats[:, 1, :]

    wy_u = mats[:, 0, 0] + mats[:, 0, 1]
    wx_u = mats[:, 1, 0] + mats[:, 1, 1]
    c_u = -(b[:, 0] + b[:, 1])
    wy_v = mats[:, 0, 0] - mats[:, 0, 1]
    wx_v = mats[:, 1, 0] - mats[:, 1, 1]
    c_v = -(b[:, 0] - b[:, 1])

    # sigma_min of each 2x2 (exact closed form).
    a_, b_, c_, d_ = mats[:, 0, 0], mats[:, 0, 1], mats[:, 1, 0], mats[:, 1, 1]
    S = a_ * a_ + b_ * b_ + c_ * c_ + d_ * d_
    D = a_ * d_ - b_ * c_
    smin = np.sqrt(np.maximum((S - np.sqrt(np.maximum(S * S - 4 * D * D, 0.0))) / 2.0, 0.0))
    reach = 2.0 / np.maximum(smin, 1e-12) + 1e-5   # small safety margin

    ys = np.linspace(-1.0, 1.0, H).astype(np.float32).astype(np.float64)
    xs = np.linspace(-WIDTH_TO_HEIGHT, WIDTH_TO_HEIGHT, W).astype(np.float32).astype(np.float64)
    gyh, gyl = _hi_lo(ys)
    gxh, gxl = _hi_lo(xs)

    wyu_h, wyu_l = _hi_lo(wy_u)
    wxu_h, wxu_l = _hi_lo(wx_u)
    cu_h, cu_l = _hi_lo(c_u)
    wyv_h, wyv_l = _hi_lo(wy_v)
    wxv_h, wxv_l = _hi_lo(wx_v)
    cv_h, cv_l = _hi_lo(c_v)

    # Per (core, tile): list of candidate point indices.
    tile_pts = [[None] * TILES_PER_CORE for _ in range(N_CORES)]
    max_cnt = 0
    for core in range(N_CORES):
        for t in range(TILES_PER_CORE):
            r, blk = divmod(t, N_BLOCKS)
            r0 = core * ROWS_PER_CORE + r * TILE_ROWS
            ylo, yhi = ys[r0], ys[r0 + TILE_ROWS - 1]
            xlo, xhi = xs[blk * TILE_COLS], xs[blk * TILE_COLS + TILE_COLS - 1]
            dy = np.maximum(np.maximum(ylo - loc[:, 0], loc[:, 0] - yhi), 0.0)
            dx = np.maximum(np.maximum(xlo - loc[:, 1], loc[:, 1] - xhi), 0.0)
            idx = np.nonzero(np.hypot(dy, dx) <= reach)[0]
            tile_pts[core][t] = idx
            max_cnt = max(max_cnt, len(idx))

    # Same program runs on every core -> chunk count per tile slot must be
    # uniform across cores.
    nchunks = [
        max(max(1, -(-len(tile_pts[c][t]) // CAP)) for c in range(N_CORES))
        for t in range(TILES_PER_CORE)
    ]
    chunk_of_tile = []   # flat chunk list: (tile_idx, chunk_idx)
    for t in range(TILES_PER_CORE):
        for k in range(nchunks[t]):
            chunk_of_tile.append((t, k))
    n_chunk = len(chunk_of_tile)

    # Packed per-core arrays.
    w_np = np.zeros((N_CORES, 8, n_chunk * 2 * CAP), np.float16)
    g_np = np.zeros((N_CORES, 8, n_chunk * TILE_PX), np.float16)
    ct_np = np.zeros((N_CORES, CAP, n_chunk * 3), np.float16)

    colf = cols.astype(np.float16)

    for core in range(N_CORES):
        for ci, (t, k) in enumerate(chunk_of_tile):
            r, blk = divmod(t, N_BLOCKS)
            r0 = core * ROWS_PER_CORE + r * TILE_ROWS
            idx = tile_pts[core][t][k * CAP:(k + 1) * CAP]
            m = len(idx)
            # Weights [8, CAP] for u at cols [2ci*CAP, ...), v next.
            o = 2 * ci * CAP
            if m:
                w_np[core, 0, o:o + m] = wyu_h[idx]
                w_np[core, 1, o:o + m] = wyu_h[idx]
                w_np[core, 2, o:o + m] = wyu_l[idx]
                w_np[core, 3, o:o + m] = wxu_h[idx]
                w_np[core, 4, o:o + m] = wxu_h[idx]
                w_np[core, 5, o:o + m] = wxu_l[idx]
                w_np[core, 6, o:o + m] = cu_h[idx]
                w_np[core, 7, o:o + m] = cu_l[idx]
                o2 = o + CAP
                w_np[core, 0, o2:o2 + m] = wyv_h[idx]
                w_np[core, 1, o2:o2 + m] = wyv_h[idx]
                w_np[core, 2, o2:o2 + m] = wyv_l[idx]
                w_np[core, 3, o2:o2 + m] = wxv_h[idx]
                w_np[core, 4, o2:o2 + m] = wxv_h[idx]
                w_np[core, 5, o2:o2 + m] = wxv_l[idx]
                w_np[core, 6, o2:o2 + m] = cv_h[idx]
                w_np[core, 7, o2:o2 + m] = cv_l[idx]
                    # -0.5 fold: device computes mapped'' = min(d,2)-2 = -2*relu(1-d/2)
                ct_np[core, :m, 3 * ci:3 * ci + 3] = (-0.5 * cols[idx]).astype(np.float16)
            # G rows [8, TILE_PX]: px = rr*TILE_COLS + col (row-major in tile)
            go = ci * TILE_PX
            ty_h = np.repeat(gyh[r0:r0 + TILE_ROWS].astype(np.float16), TILE_COLS)
            ty_l = np.repeat(gyl[r0:r0 + TILE_ROWS].astype(np.float16), TILE_COLS)
            tx_h = np.tile(gxh[blk * TILE_COLS:(blk + 1) * TILE_COLS].astype(np.float16), TILE_ROWS)
            tx_l = np.tile(gxl[blk * TILE_COLS:(blk + 1) * TILE_COLS].astype(np.float16), TILE_ROWS)
            g_np[core, 0, go:go + TILE_PX] = ty_h
            g_np[core, 1, go:go + TILE_PX] = ty_l
            g_np[core, 2, go:go + TILE_PX] = ty_h
            g_np[core, 3, go:go + TILE_PX] = tx_h
            g_np[core, 4, go:go + TILE_PX] = tx_l
            g_np[core, 5, go:go + TILE_PX] = tx_h
            g_np[core, 6, go:go + TILE_PX] = 1.0
            g_np[core, 7, go:go + TILE_PX] = 1.0

    return w_np, g_np, ct_np, chunk_of_tile, n_chunk


def _build_nc(n_chunk, chunk_of_tile):
    """Build the Bass/Tile program (shared by all cores)."""
    from contextlib import ExitStack

    import concourse.bacc as bacc
    import concourse.tile as tile
    from concourse import mybir

    f16 = mybir.dt.float16
    f32 = mybir.dt.float32
    nc = bacc.Bacc("TRN2", target_bir_lowering=False, debug=False,
                   num_devices=N_CORES)

    w_d = nc.dram_tensor("w", [8, n_chunk * 2 * CAP], f16, kind="ExternalInput")
    g_d = nc.dram_tensor("g", [8, n_chunk * TILE_PX], f16, kind="ExternalInput")
    ct_d = nc.dram_tensor("ct", [CAP, n_chunk * 3], f16, kind="ExternalInput")
    y_d = nc.dram_tensor("y", [128, ROWS_PER_CORE * N_BLOCKS * 3], f32, kind="ExternalOutput")

    with ExitStack() as ctx:
        tc = ctx.enter_context(tile.TileContext(nc))
        const = ctx.enter_context(tc.tile_pool(name="const", bufs=1))
        uvpool = ctx.enter_context(tc.tile_pool(name="uv", bufs=3, space="PSUM"))
        cvpool = ctx.enter_context(tc.tile_pool(name="cv", bufs=1, space="PSUM"))
        dpool = ctx.enter_context(tc.tile_pool(name="d", bufs=3))
        mpool = ctx.enter_context(tc.tile_pool(name="m", bufs=3))
        opool = ctx.enter_context(tc.tile_pool(name="o", bufs=1))

        W_sb = const.tile([8, n_chunk * 2 * CAP], f16)
        G_sb = const.tile([8, n_chunk * TILE_PX], f16)
        CT_sb = const.tile([CAP, n_chunk * 3], f16)
        nc.sync.dma_start(W_sb[:], w_d[:])
        nc.sync.dma_start(G_sb[:], g_d[:])
        nc.sync.dma_start(CT_sb[:], ct_d[:])

        canvas = cvpool.tile([128, ROWS_PER_CORE * N_BLOCKS * 3], f32)

        # chunk index ranges per tile for start/stop flags
        first_chunk = {}
        last_chunk = {}
        for ci, (t, k) in enumerate(chunk_of_tile):
            first_chunk.setdefault(t, ci)
            last_chunk[t] = ci

        for ci, (t, k) in enumerate(chunk_of_tile):
            r, blk = divmod(t, N_BLOCKS)
            puv = uvpool.tile([128, 2 * TILE_PX], f32, tag="uv")
            wo = 2 * ci * CAP
            go = ci * TILE_PX
            nc.tensor.matmul(puv[:, 0:TILE_PX], W_sb[:, wo:wo + CAP],
                             G_sb[:, go:go + TILE_PX], start=True, stop=True)
            nc.tensor.matmul(puv[:, TILE_PX:2 * TILE_PX],
                             W_sb[:, wo + CAP:wo + 2 * CAP],
                             G_sb[:, go:go + TILE_PX], start=True, stop=True)
            # |u| and |v| in one ACT op (single 2-bank PSUM read)
            aa_sb = dpool.tile([128, 2 * TILE_PX], f16, tag="aa")
            nc.scalar.activation(aa_sb[:], puv[:],
                                 mybir.ActivationFunctionType.Abs)
            d_sb = dpool.tile([128, TILE_PX], f16, tag="d")
            nc.vector.tensor_tensor(d_sb[:], aa_sb[:, 0:TILE_PX],
                                    aa_sb[:, TILE_PX:2 * TILE_PX],
                                    op=mybir.AluOpType.max)
            m_sb = mpool.tile([128, TILE_PX], f16, tag="m")
            nc.vector.tensor_scalar(
                m_sb[:], d_sb[:], 2.0, 2.0,
                op0=mybir.AluOpType.min, op1=mybir.AluOpType.subtract)
            for rr in range(TILE_ROWS):
                lr = r * TILE_ROWS + rr
                off = 3 * (lr * N_BLOCKS + blk)
                nc.tensor.matmul(canvas[:, off:off + 3],
                                 m_sb[:, rr * TILE_COLS:(rr + 1) * TILE_COLS],
                                 CT_sb[:, 3 * ci:3 * ci + 3],
                                 start=(ci == first_chunk[t]),
                                 stop=(ci == last_chunk[t]))

        out_sb = opool.tile([128, ROWS_PER_CORE * N_BLOCKS * 3], f32)
        nc.scalar.activation(out_sb[:], canvas[:],
                             mybir.ActivationFunctionType.Sigmoid, scale=4.0)
        nc.sync.dma_start(y_d[:], out_sb[:])

    nc.compile()
    return nc


def _install_ntff_hook():
    """Provide antenv.axon_hooks if the image lacks it (ctypes shim around
    libaxon_pjrt.so's NRT profile capture). Returns True on success."""
    try:
        from antenv.axon_hooks import get_axon_ntff_profile_hook  # noqa: F401
        return True
    except ImportError:
        pass
    try:
        import contextlib
        import ctypes
        import types

        import antenv

        so_path = "/opt/axon/libaxon_pjrt.so"
        lib = ctypes.CDLL(so_path)
        if not hasattr(lib, "axon_start_nrt_profile"):
            return False
        lib.axon_start_nrt_profile.argtypes = [
            ctypes.POINTER(ctypes.c_int64), ctypes.c_size_t]
        lib.axon_start_nrt_profile.restype = ctypes.c_int64
        lib.axon_stop_nrt_profile.argtypes = [ctypes.c_char_p]
        lib.axon_stop_nrt_profile.restype = ctypes.c_int64

        @contextlib.contextmanager
        def _hook(output_dir, device_ids):
            import jax
            jax.devices()
            if device_ids:
                ids = (ctypes.c_int64 * len(device_ids))(*device_ids)
                rc = lib.axon_start_nrt_profile(ids, len(device_ids))
            else:
                rc = lib.axon_start_nrt_profile(None, 0)
            if rc != 0:
                raise RuntimeError(f"axon_start_nrt_profile rc={rc}")
            try:
                yield
            finally:
                n = lib.axon_stop_nrt_profile(str(output_dir).encode())
                print(f"ntff profile: {n} file(s) -> {output_dir}", file=sys.stderr)

        mod = types.ModuleType("antenv.axon_hooks")
        mod._hook = _hook
        mod.get_axon_ntff_profile_hook = lambda: _hook
        mod.set_axon_ntff_profile_hook = lambda h: None
        sys.modules["antenv.axon_hooks"] = mod
        antenv.axon_hooks = mod
        return True
    except Exception as e:  # pragma: no cover
        print("ntff hook install failed:", e, file=sys.stderr)
        return False


def kernel(locations, matrix_offsets, matrix_scale_exponents, colors,
           canvas_height_px, canvas_width_px):
    assert int(canvas_height_px) == H and int(canvas_width_px) == W

    w_np, g_np, ct_np, chunk_of_tile, n_chunk = _prepare(
        locations, matrix_offsets, matrix_scale_exponents, colors)

    nc = _build_nc(n_chunk, chunk_of_tile)

    from concourse.bass_utils import run_bass_kernel_spmd

    in_maps = [
        {"w": w_np[c], "g": g_np[c], "ct": ct_np[c]} for c in range(N_CORES)
    ]
    trace = bool(int(os.environ.get("BASS_KERNEL_TRACE", "0")))
    if trace:
        trace = _install_ntff_hook()
    try:
        res = run_bass_kernel_spmd(nc, in_maps, core_ids=list(range(N_CORES)),
                                   trace=trace)
    except Exception:
        if not trace:
            raise
        res = run_bass_kernel_spmd(nc, in_maps, core_ids=list(range(N_CORES)),
                                   trace=False)
    last_run_info.clear()
    last_run_info.update(
        exec_time_ns=res.exec_time_ns,
        mean_exec_time_ns=res.mean_exec_time_ns,
        profile_json=res.profile_json,
    )

    out = np.empty((3, H, W), np.float32)
    for c in range(N_CORES):
        y = res.results[c]["y"]                       # [128, 32*3*3]
        arr = y.reshape(128, ROWS_PER_CORE, N_BLOCKS, 3)  # p, lr, blk, ch
        out[:, c * ROWS_PER_CORE:(c + 1) * ROWS_PER_CORE, :] = (
            arr.transpose(3, 1, 2, 0).reshape(3, ROWS_PER_CORE, W))
    return out



# revision 23
# speedup vs baseline: 1.0665x; 1.0665x over previous
"""Trainium2 Bass kernel for the nn_Points problem (v2).

Renders N=1024 anisotropic "diamond" points onto a 3x256x384 canvas:
    t = (pixel - loc) @ M_n          (2-vector per pixel per point)
    mapped = relu(1 - (|t0|+|t1|)/2)
    canvas = sigmoid(4 * sum_n mapped * color_n)

v2 design (8 NeuronCores, full inputs in / full output out):
  * Spatial-shard the canvas: core c renders rows [32c, 32c+32).
  * 12 tiles per core of 16 rows x 64 cols (1024 px).  Exact SAT
    culling (rect vs. preimage of the |t|_1<=2 diamond) keeps every
    tile's candidate-point count <= 64 (measured max 55).
  * u = t0+t1, v = t0-t1 are affine in (gy, gx).  ONE matmul per tile
    computes both: stationary W [8, 128] has u-weights in cols 0:64
    and v-weights in cols 64:128; moving G [8, 1024] is the hi/lo
    fp16-split pixel grid -> PSUM [128, 1024] f32 (u rows 0:64).
  * |u|,|v| via one ACT Abs per tile (the single PSUM pass the HW
    allows; DVE may read at most one PSUM operand), then d =
    max(|u|,|v|) = |t0|+|t1| as a DVE max over the fp16 SBUF halves.
  * mapped'' = min(d,2)-2 (= -2*mapped) per tile PAIR in one DVE
    tensor_scalar; colors are pre-scaled by -c/2.
  * canvas: one matmul per pair with block-diagonal stationary
    [ctA | 0; 0 | ctB] [128, 6] and mapped' [128, 1024] moving ->
    out [6, 1024] written at PSUM partition base {0,32,64,96} of one
    of two persistent [128, 1024] canvas regions.
  * One ACT sigmoid(4x) per canvas region, DMA out [6, 1024] slices.
"""

import math
import os
import sys

import numpy as np

for _p in ("/opt/trn_rl_repo",):
    if _p not in sys.path and os.path.isdir(_p):
        sys.path.insert(0, _p)

# Geometry (matches the reference module's fixed canvas).
H, W = 256, 384
N_CORES = 8
ROWS_PER_CORE = H // N_CORES            # 32
TILE_ROWS, TILE_COLS = 16, 64           # spatial tile = 1024 px
N_BANDS = ROWS_PER_CORE // TILE_ROWS    # 2 row-bands per core
N_BLOCKS = W // TILE_COLS               # 6 col-blocks per row-band
TILES_PER_CORE = N_BANDS * N_BLOCKS     # 12
N_PAIRS = TILES_PER_CORE // 2           # 6
TILE_PX = TILE_ROWS * TILE_COLS         # 1024
CAP = 64                                # points per tile (u|v fused)
WIDTH_TO_HEIGHT = 384.0 / 256.0

# Set BASS_KERNEL_TRACE=1 to capture an NTFF profile; results land here.
last_run_info = {}


def _hi_lo(x):
    """Split float64 array into fp16 hi + fp16 lo with tiny residual."""
    hi = x.astype(np.float16)
    lo = (x - hi.astype(np.float64)).astype(np.float16)
    return hi, lo


def _prepare(locations, matrix_offsets, matrix_scale_exponents, colors):
    """Host-side prep: per-point affine combos, SAT culling, packing."""
    loc = np.asarray(locations, np.float64).reshape(-1, 2)      # (N, 2) y,x
    mo = np.asarray(matrix_offsets, np.float64)                  # (N, 2, 2)
    mse = np.asarray(matrix_scale_exponents, np.float64).reshape(-1)
    cols = np.asarray(colors, np.float64).reshape(-1, 3)         # (N, 3)
    n = loc.shape[0]

    scale = (math.sqrt(n) / 2.0) / np.exp(mse)
    mats = mo + np.eye(2)[None, :, :] * scale[:, None, None]     # (N, 2, 2)
    # b_j = loc_y*M[0,j] + loc_x*M[1,j]
    b = loc[:, 0, None] * mats[:, 0, :] + loc[:, 1, None] * mats[:, 1, :]

    wy_u = mats[:, 0, 0] + mats[:, 0, 1]
    wx_u = mats[:, 1, 0] + mats[:, 1, 1]
    c_u = -(b[:, 0] + b[:, 1])
    wy_v = mats[:, 0, 0] - mats[:, 0, 1]
    wx_v = mats[:, 1, 0] - mats[:, 1, 1]
    c_v = -(b[:, 0] - b[:, 1])

    # Exact SAT cull: tile rect intersects {|u|<=2, |v|<=2} iff all four
    # separating-axis interval tests pass (y, x, u, v axes).
    det = wy_u * wx_v - wx_u * wy_v
    A00 = wx_v / det
    A01 = -wx_u / det
    A10 = -wy_v / det
    A11 = wy_u / det
    y0 = A00 * (-c_u) + A01 * (-c_v)
    x0 = A10 * (-c_u) + A11 * (-c_v)
    hy = 2 * (np.abs(A00) + np.abs(A01))
    hx = 2 * (np.abs(A10) + np.abs(A11))

    ys = np.linspace(-1.0, 1.0, H).astype(np.float32).astype(np.float64)
    xs = np.linspace(-WIDTH_TO_HEIGHT, WIDTH_TO_HEIGHT, W).astype(np.float32).astype(np.float64)
    gyh, gyl = _hi_lo(ys)
    gxh, gxl = _hi_lo(xs)

    wyu_h, wyu_l = _hi_lo(wy_u)
    wxu_h, wxu_l = _hi_lo(wx_u)
    cu_h, cu_l = _hi_lo(c_u)
    wyv_h, wyv_l = _hi_lo(wy_v)
    wxv_h, wxv_l = _hi_lo(wx_v)
    cv_h, cv_l = _hi_lo(c_v)

    w_np = np.zeros((N_CORES, 8, TILES_PER_CORE * 2 * CAP), np.float16)
    g_np = np.zeros((N_CORES, 8, TILES_PER_CORE * TILE_PX), np.float16)
    # Canvas stationary is padded to 32 output rows per pair (cols 6:32
    # zero) so each matmul initializes a full 32-partition PSUM slot.
    ct_np = np.zeros((N_CORES, 128, N_PAIRS * 32), np.float16)

    for core in range(N_CORES):
        for t in range(TILES_PER_CORE):
            rb, cb = divmod(t, N_BLOCKS)
            r0 = core * ROWS_PER_CORE + rb * TILE_ROWS
            c0 = cb * TILE_COLS
            ylo, yhi = ys[r0], ys[r0 + TILE_ROWS - 1]
            xlo, xhi = xs[c0], xs[c0 + TILE_COLS - 1]
            yc, xc = (ylo + yhi) / 2, (xlo + xhi) / 2
            ry, rx = (yhi - ylo) / 2, (xhi - xlo) / 2
            ok_y = np.abs(yc - y0) <= ry + hy + 1e-9
            ok_x = np.abs(xc - x0) <= rx + hx + 1e-9
            uc = wy_u * yc + wx_u * xc + c_u
            du = np.abs(wy_u) * ry + np.abs(wx_u) * rx
            ok_u = np.abs(uc) <= 2 + du + 1e-9
            vc = wy_v * yc + wx_v * xc + c_v
            dv = np.abs(wy_v) * ry + np.abs(wx_v) * rx
            ok_v = np.abs(vc) <= 2 + dv + 1e-9
            idx = np.nonzero(ok_y & ok_x & ok_u & ok_v)[0]
            m = len(idx)
            assert m <= CAP, f"tile candidate overflow: {m} > {CAP}"

            # Stationary W [8, 128]: cols 0:64 u-weights, 64:128 v-weights.
            # Row structure pairs with G rows [yh, yl, yh, xh, xl, xh, 1, 1].
            o = 2 * t * CAP
            if m:
                w_np[core, 0, o:o + m] = wyu_h[idx]
                w_np[core, 1, o:o + m] = wyu_h[idx]
                w_np[core, 2, o:o + m] = wyu_l[idx]
                w_np[core, 3, o:o + m] = wxu_h[idx]
                w_np[core, 4, o:o + m] = wxu_h[idx]
                w_np[core, 5, o:o + m] = wxu_l[idx]
                w_np[core, 6, o:o + m] = cu_h[idx]
                w_np[core, 7, o:o + m] = cu_l[idx]
                o2 = o + CAP
                w_np[core, 0, o2:o2 + m] = wyv_h[idx]
                w_np[core, 1, o2:o2 + m] = wyv_h[idx]
                w_np[core, 2, o2:o2 + m] = wyv_l[idx]
                w_np[core, 3, o2:o2 + m] = wxv_h[idx]
                w_np[core, 4, o2:o2 + m] = wxv_h[idx]
                w_np[core, 5, o2:o2 + m] = wxv_l[idx]
                w_np[core, 6, o2:o2 + m] = cv_h[idx]
                w_np[core, 7, o2:o2 + m] = cv_l[idx]
                # mapped'' = min(d,2)-2 = -2*mapped -> colors scaled by -c/2.
                p, s01 = divmod(t, 2)
                ct_np[core, 64 * s01:64 * s01 + m, 32 * p + 3 * s01:32 * p + 3 * s01 + 3] = (
                    -0.5 * cols[idx]).astype(np.float16)

            # Moving G [8, TILE_PX]: px = rr*TILE_COLS + cc (row-major).
            go = t * TILE_PX
            ty_h = np.repeat(gyh[r0:r0 + TILE_ROWS], TILE_COLS)
            ty_l = np.repeat(gyl[r0:r0 + TILE_ROWS], TILE_COLS)
            tx_h = np.tile(gxh[c0:c0 + TILE_COLS], TILE_ROWS)
            tx_l = np.tile(gxl[c0:c0 + TILE_COLS], TILE_ROWS)
            g_np[core, 0, go:go + TILE_PX] = ty_h
            g_np[core, 1, go:go + TILE_PX] = ty_l
            g_np[core, 2, go:go + TILE_PX] = ty_h
            g_np[core, 3, go:go + TILE_PX] = tx_h
            g_np[core, 4, go:go + TILE_PX] = tx_l
            g_np[core, 5, go:go + TILE_PX] = tx_h
            g_np[core, 6, go:go + TILE_PX] = 1.0
            g_np[core, 7, go:go + TILE_PX] = 1.0

    return w_np, g_np, ct_np


def _build_nc():
    """Build the Bass/Tile program (shared by all cores)."""
    from contextlib import ExitStack

    import concourse.bacc as bacc
    import concourse.tile as tile
    from concourse import mybir

    f16 = mybir.dt.float16
    f32 = mybir.dt.float32
    nc = bacc.Bacc("TRN2", target_bir_lowering=False, debug=False,
                   num_devices=N_CORES)

    w_d = nc.dram_tensor("w", [8, TILES_PER_CORE * 2 * CAP], f16, kind="ExternalInput")
    g_d = nc.dram_tensor("g", [8, TILES_PER_CORE * TILE_PX], f16, kind="ExternalInput")
    ct_d = nc.dram_tensor("ct", [128, N_PAIRS * 32], f16, kind="ExternalInput")
    y_d = nc.dram_tensor("y", [6 * N_PAIRS, TILE_PX], f32, kind="ExternalOutput")

    with ExitStack() as ctx:
        tc = ctx.enter_context(tile.TileContext(nc))
        const = ctx.enter_context(tc.tile_pool(name="const", bufs=1))
        uvpool = ctx.enter_context(tc.tile_pool(name="uv", bufs=2, space="PSUM"))
        cvpool = ctx.enter_context(tc.tile_pool(name="cv", bufs=1, space="PSUM"))
        apool = ctx.enter_context(tc.tile_pool(name="a", bufs=3))
        mpool = ctx.enter_context(tc.tile_pool(name="m", bufs=2))
        rpool = ctx.enter_context(tc.tile_pool(name="r", bufs=2))
        opool = ctx.enter_context(tc.tile_pool(name="o", bufs=1))

        W_sb = const.tile([8, TILES_PER_CORE * 2 * CAP], f16)
        G_sb = const.tile([8, TILES_PER_CORE * TILE_PX], f16)
        CT_sb = const.tile([128, N_PAIRS * 32], f16)
        nc.sync.dma_start(W_sb[:], w_d[:])
        nc.sync.dma_start(CT_sb[:], ct_d[:])
        nc.sync.dma_start(G_sb[:], g_d[:])

        # Two persistent canvas regions; pair p -> region p//3, slot p%3
        # (partition base 32*slot, rows +0:6 hold [tileA ch | tileB ch]).
        canvas0 = cvpool.tile([128, TILE_PX], f32)
        canvas1 = cvpool.tile([128, TILE_PX], f32)
        regions = [canvas0, canvas1]

        for p in range(N_PAIRS):
            dm = mpool.tile([128, TILE_PX], f16, tag="m")
            sh = mpool.tile([128, TILE_PX], f16, tag="sh")
            for s01 in range(2):
                t = 2 * p + s01
                puv = uvpool.tile([128, TILE_PX], f32, tag="uv")
                wo = 2 * t * CAP
                go = t * TILE_PX
                # PSUM bank = 512 f32: one matmul per 512-col half.
                for h in range(0, TILE_PX, 512):
                    nc.tensor.matmul(puv[:, h:h + 512], W_sb[:, wo:wo + 2 * CAP],
                                     G_sb[:, go + h:go + h + 512],
                                     start=True, stop=True)
                # |u|,|v|: the single allowed PSUM pass, on the ACT engine.
                ab = apool.tile([128, TILE_PX], f16, tag="ab")
                nc.scalar.activation(ab[:], puv[:],
                                     mybir.ActivationFunctionType.Abs)
                # DVE requires equal SB base partitions, so shift the
                # opposite half onto this tile's rows via a DMA engine.
                b0, b1 = (0, 64) if s01 == 0 else (64, 0)
                nc.sync.dma_start(sh[b0:b0 + 64, :], ab[b1:b1 + 64, :])
                # d = max(|u|,|v|), base-aligned.
                nc.vector.tensor_tensor(dm[b0:b0 + 64, :],
                                        ab[b0:b0 + 64, :], sh[b0:b0 + 64, :],
                                        op=mybir.AluOpType.max)
            # mapped'' = min(d,2)-2 for both tiles of the pair.
            mr = rpool.tile([128, TILE_PX], f16, tag="mr")
            nc.vector.tensor_scalar(
                mr[:], dm[:], 2.0, 2.0,
                op0=mybir.AluOpType.min, op1=mybir.AluOpType.subtract)
            reg = regions[p // 3]
            s = p % 3
            for h in range(0, TILE_PX, 512):
                nc.tensor.matmul(reg[32 * s:32 * s + 32, h:h + 512],
                                 CT_sb[:, 32 * p:32 * p + 32], mr[:, h:h + 512],
                                 start=True, stop=True)

        for ri in range(2):
            n_here = min(3, N_PAIRS - 3 * ri)
            if n_here <= 0:
                continue
            outr = opool.tile([128, TILE_PX], f32, tag="out")
            nc.scalar.activation(outr[0:32 * n_here, :],
                                 regions[ri][0:32 * n_here, :],
                                 mybir.ActivationFunctionType.Sigmoid,
                                 scale=4.0)
            for s in range(n_here):
                p = 3 * ri + s
                nc.sync.dma_start(y_d[6 * p:6 * p + 6, :],
                                  outr[32 * s:32 * s + 6, :])

    nc.compile()
    return nc


def _install_ntff_hook():
    """Provide antenv.axon_hooks if the image lacks it (ctypes shim around
    libaxon_pjrt.so's NRT profile capture). Returns True on success."""
    try:
        from antenv.axon_hooks import get_axon_ntff_profile_hook  # noqa: F401
        return True
    except ImportError:
        pass
    try:
        import contextlib
        import ctypes
        import types

        import antenv

        so_path = "/opt/axon/libaxon_pjrt.so"
        lib = ctypes.CDLL(so_path)
        if not hasattr(lib, "axon_start_nrt_profile"):
            return False
        lib.axon_start_nrt_profile.argtypes = [
            ctypes.POINTER(ctypes.c_int64), ctypes.c_size_t]
        lib.axon_start_nrt_profile.restype = ctypes.c_int64
        lib.axon_stop_nrt_profile.argtypes = [ctypes.c_char_p]
        lib.axon_stop_nrt_profile.restype = ctypes.c_int64

        @contextlib.contextmanager
        def _hook(output_dir, device_ids):
            import jax
            jax.devices()
            if device_ids:
                ids = (ctypes.c_int64 * len(device_ids))(*device_ids)
                rc = lib.axon_start_nrt_profile(ids, len(device_ids))
            else:
                rc = lib.axon_start_nrt_profile(None, 0)
            if rc != 0:
                raise RuntimeError(f"axon_start_nrt_profile rc={rc}")
            try:
                yield
            finally:
                n = lib.axon_stop_nrt_profile(str(output_dir).encode())
                print(f"ntff profile: {n} file(s) -> {output_dir}", file=sys.stderr)

        mod = types.ModuleType("antenv.axon_hooks")
        mod._hook = _hook
        mod.get_axon_ntff_profile_hook = lambda: _hook
        mod.set_axon_ntff_profile_hook = lambda h: None
        sys.modules["antenv.axon_hooks"] = mod
        antenv.axon_hooks = mod
        return True
    except Exception as e:  # pragma: no cover
        print("ntff hook install failed:", e, file=sys.stderr)
        return False


def _unshard(results):
    """Reassemble per-core y [36, 1024] into the full (3, H, W) canvas."""
    out = np.empty((3, H, W), np.float32)
    for c in range(N_CORES):
        y = results[c]["y"]                                  # [36, 1024]
        for t in range(TILES_PER_CORE):
            p, s01 = divmod(t, 2)
            rb, cb = divmod(t, N_BLOCKS)
            blk = y[6 * p + 3 * s01:6 * p + 3 * s01 + 3, :]  # [3, 1024]
            r0 = c * ROWS_PER_CORE + rb * TILE_ROWS
            out[:, r0:r0 + TILE_ROWS, cb * TILE_COLS:(cb + 1) * TILE_COLS] = (
                blk.reshape(3, TILE_ROWS, TILE_COLS))
    return out


def kernel(locations, matrix_offsets, matrix_scale_exponents, colors,
           canvas_height_px, canvas_width_px):
    assert int(canvas_height_px) == H and int(canvas_width_px) == W

    w_np, g_np, ct_np = _prepare(
        locations, matrix_offsets, matrix_scale_exponents, colors)

    nc = _build_nc()

    from concourse.bass_utils import run_bass_kernel_spmd

    in_maps = [
        {"w": w_np[c], "g": g_np[c], "ct": ct_np[c]} for c in range(N_CORES)
    ]
    trace = bool(int(os.environ.get("BASS_KERNEL_TRACE", "0")))
    if trace:
        trace = _install_ntff_hook()
    try:
        res = run_bass_kernel_spmd(nc, in_maps, core_ids=list(range(N_CORES)),
                                   trace=trace)
    except Exception:
        if not trace:
            raise
        res = run_bass_kernel_spmd(nc, in_maps, core_ids=list(range(N_CORES)),
                                   trace=False)
    last_run_info.clear()
    last_run_info.update(
        exec_time_ns=res.exec_time_ns,
        mean_exec_time_ns=res.mean_exec_time_ns,
        profile_json=res.profile_json,
    )

    return _unshard(res.results)
